# revision 3
# baseline (speedup 1.0000x reference)
"""Trainium2 Bass kernel for nn_PhysicsForwardModel — table-factorized version.

Math (reference):
  W[b]       = A2 @ x_b @ Bm^T         A2 = diag(v) @ Dy[:, :NZ] / 3
  att[t,i,j] = exp(-chi*F[i,j]*t) * cos(F[i,j]*t)
  out[b,t,j] = sum_i att[t,i,j] * W[b,i,j]

Split t = S*k + s (S=64, k<8).  Angle addition gives
  att[t] = Ck * cs_s - Sk * ss_s
with x-independent tables (precomputed on host, fp16, DMA'd in):
  cs[i,j,s] = e^{-chi F s} cos(F s)     ss[i,j,s] = e^{-chi F s} sin(F s)
  Ck[i,j]   = e^{-chi F S k} cos(F S k) Sk[i,j]  = -e^{-chi F S k} sin(F S k)
Folding the k-level into the matmul stationary:
  out[b, S*k+s, j] = sum_i (W*Ck)[b,i,j] cs[i,j,s] + sum_i (W*Sk)[b,i,j] ss[i,j,s]
so the device does: W build (PE), WC/WS = W*coeff (DVE, fp16), then per
(c-chunk, j) two accumulating matmuls with stationary (128, 8k*4b) and moving
table slice (128, 64) — all transcendentals are gone from the device.

Sharding: j (=LX) split across 8 cores, 64 columns each.  Tables are DMA'd in
j-slices so the matmul loop pipelines under the table transfer (the kernel is
HBM-bandwidth-bound on the ~9.5MB/core of tables); the final slices are
smaller to shorten the end-of-stream tail.
Output per core: per-slice (32=(k,b), jq, 64 s) f32; host unscrambles.
"""

import numpy as np

BATCH, NZ, NX = 4, 128, 128
LY = LX = 512
T = LY
NCORES = 8
JPER = LX // NCORES   # 64
NCHUNK = LY // 128    # 4 i-chunks
S = 64                # timesteps per block
KBLK = T // S         # 8 blocks
# j-slice sizes for DMA/compute pipelining; final slices smaller to cut the
# end-of-stream latency tail.  One PSUM group per slice.
SLICES = [8, 8, 8, 8, 8, 8, 8, 4, 4]
SOFF = [sum(SLICES[:i]) for i in range(len(SLICES))]
NQ = len(SLICES)
CHI = float(np.float32(0.03))

_cache = {}


def _dct_mat(N):
    n = np.arange(N, dtype=np.float64)
    D = np.cos(np.pi * (2.0 * n[None, :] + 1.0) * n[:, None] / (2.0 * N))
    s = np.where(np.arange(N) == 0, np.sqrt(1.0 / N), np.sqrt(2.0 / N))
    return s[:, None] * D


def _host_constants():
    Dy = _dct_mat(LY)
    Dx = _dct_mat(LX)
    A = Dy[:, :NZ]                      # (LY, NZ)
    v = Dy @ Dy[:, 0]                   # (LY,)
    A2 = (v[:, None] * A) / 3.0         # (LY, NZ)
    Bm = Dx[:, NX:2 * NX]               # (LX, NX)
    kx = np.arange(LX, dtype=np.float64) / LX * np.pi
    ky = np.arange(LY, dtype=np.float64) / LY * np.pi
    F = np.sqrt(kx[None, :] ** 2 + ky[:, None] ** 2)  # (LY i, LX j)

    aT = A2.T.astype(np.float32)        # (128 n, 512 i)
    bT = Bm.T.astype(np.float32)        # (128 m, 512 j)

    # per-(i,j) tables; i -> (c, p) with i = 128c + p
    s_ = np.arange(S, dtype=np.float64)
    tk = np.arange(KBLK, dtype=np.float64) * S
    Fc = F.reshape(NCHUNK, 128, LX).transpose(1, 0, 2)     # (128 p, 4 c, 512 j)
    cs = np.exp(-CHI * Fc[..., None] * s_) * np.cos(Fc[..., None] * s_)
    ss = np.exp(-CHI * Fc[..., None] * s_) * np.sin(Fc[..., None] * s_)
    ck = np.exp(-CHI * Fc[..., None] * tk) * np.cos(Fc[..., None] * tk)
    sk = -np.exp(-CHI * Fc[..., None] * tk) * np.sin(Fc[..., None] * tk)
    # cs/ss: (128, 4, 512 j, 64 s); ck/sk -> (128, 4, 8 k, 512 j)
    ck = ck.transpose(0, 1, 3, 2)
    sk = sk.transpose(0, 1, 3, 2)
    return (aT, bT,
            cs.astype(np.float16), ss.astype(np.float16),
            ck.astype(np.float16), sk.astype(np.float16))


def _build_program():
    import concourse.tile as tile
    from concourse import bacc, mybir

    f32 = mybir.dt.float32
    f16 = mybir.dt.float16

    nc = bacc.Bacc("TRN2", target_bir_lowering=False, debug=False)

    x_d = nc.dram_tensor("x", (NZ, BATCH, NX), f16, kind="ExternalInput").ap()
    aT_d = nc.dram_tensor("aT", (NZ, LY), f16, kind="ExternalInput").ap()
    bT_d = nc.dram_tensor("bT", (NX, JPER), f16, kind="ExternalInput").ap()
    ck_d = nc.dram_tensor("ck", (128, NCHUNK, KBLK, JPER), f16,
                          kind="ExternalInput").ap()
    sk_d = nc.dram_tensor("sk", (128, NCHUNK, KBLK, JPER), f16,
                          kind="ExternalInput").ap()
    cs_d = [nc.dram_tensor(f"cs{q}", (128, NCHUNK, SLICES[q], S), f16,
                           kind="ExternalInput").ap() for q in range(NQ)]
    ss_d = [nc.dram_tensor(f"ss{q}", (128, NCHUNK, SLICES[q], S), f16,
                           kind="ExternalInput").ap() for q in range(NQ)]
    # out{q}[p=(4k+b), j_in_slice, s] -> host unscrambles
    out_d = [nc.dram_tensor(f"out{q}", (KBLK * BATCH, SLICES[q], S), f32,
                            kind="ExternalOutput").ap() for q in range(NQ)]

    with tile.TileContext(nc) as tc:
        with tc.tile_pool(name="singles", bufs=1) as singles, \
             tc.tile_pool(name="wbuild", bufs=2) as wbuild, \
             tc.tile_pool(name="outp", bufs=8) as outp, \
             tc.tile_pool(name="ps_w", bufs=1, space="PSUM") as ps_w, \
             tc.tile_pool(name="ps_main", bufs=6, space="PSUM") as ps_main:

            # ---- input DMAs: big table stream first, smalls interleaved ----
            x_sb = singles.tile([NZ, BATCH, NX], f16)
            aT_sb = singles.tile([NZ, LY], f16)
            bT_sb = singles.tile([NX, JPER], f16)
            ck_sb = singles.tile([128, NCHUNK, KBLK, JPER], f16)
            sk_sb = singles.tile([128, NCHUNK, KBLK, JPER], f16)
            cs_sb = [singles.tile([128, NCHUNK, SLICES[q], S], f16,
                                  name=f"cs{q}") for q in range(NQ)]
            ss_sb = [singles.tile([128, NCHUNK, SLICES[q], S], f16,
                                  name=f"ss{q}") for q in range(NQ)]
            nc.sync.dma_start(x_sb[:], x_d)
            nc.sync.dma_start(aT_sb[:], aT_d)
            nc.sync.dma_start(bT_sb[:], bT_d)
            nc.sync.dma_start(cs_sb[0][:], cs_d[0])
            nc.sync.dma_start(ss_sb[0][:], ss_d[0])
            nc.sync.dma_start(ck_sb[:], ck_d)
            nc.sync.dma_start(sk_sb[:], sk_d)
            for q in range(1, NQ):
                nc.sync.dma_start(cs_sb[q][:], cs_d[q])
                nc.sync.dma_start(ss_sb[q][:], ss_d[q])

            # ---- W build: w_c[c][p_i, b, j] = W[b, 128c+p_i, j] (fp16) ----
            w_c = [singles.tile([128, BATCH, JPER], f16, name=f"w{c}")
                   for c in range(NCHUNK)]
            for b in range(BATCH):
                mm1_ps = ps_w.tile([128, LY], f32, tag="mm1")
                # mm1_ps[m, i] = sum_n x[b,n,m] * aT[n,i]  ( = (A2 @ x_b)^T )
                nc.tensor.matmul(mm1_ps[:], x_sb[:, b, :], aT_sb[:],
                                 start=True, stop=True)
                mm1_sb = wbuild.tile([128, LY], f16)
                nc.scalar.copy(mm1_sb[:], mm1_ps[:])
                for c in range(NCHUNK):
                    w_ps = ps_w.tile([128, JPER], f32, tag="wps")
                    # w_ps[i, j] = sum_m mm1_sb[m, 128c+i] * bT[m, j]
                    nc.tensor.matmul(w_ps[:], mm1_sb[:, c * 128:(c + 1) * 128],
                                     bT_sb[:], start=True, stop=True)
                    nc.vector.tensor_copy(w_c[c][:, b, :], w_ps[:])

            # ---- stationaries, per (c, j-quarter) so PE can start early ----
            # wc[c][h][p, k, b, jj] = w * ck ;  ws = w * sk
            JH = JPER // 4  # 16
            wc_t = [[singles.tile([128, KBLK, BATCH, JH], f16, name=f"wc{c}_{h}")
                     for h in range(4)] for c in range(NCHUNK)]
            ws_t = [[singles.tile([128, KBLK, BATCH, JH], f16, name=f"ws{c}_{h}")
                     for h in range(4)] for c in range(NCHUNK)]
            for h in range(4):
                jsl = slice(h * JH, (h + 1) * JH)
                for c in range(NCHUNK):
                    nc.vector.tensor_tensor(
                        wc_t[c][h][:],
                        w_c[c][:, None, :, jsl].to_broadcast(
                            (128, KBLK, BATCH, JH)),
                        ck_sb[:, c, :, None, jsl].to_broadcast(
                            (128, KBLK, BATCH, JH)),
                        mybir.AluOpType.mult)
                    nc.vector.tensor_tensor(
                        ws_t[c][h][:],
                        w_c[c][:, None, :, jsl].to_broadcast(
                            (128, KBLK, BATCH, JH)),
                        sk_sb[:, c, :, None, jsl].to_broadcast(
                            (128, KBLK, BATCH, JH)),
                        mybir.AluOpType.mult)

            # ---- main loop: per j, 8 accumulating matmuls into PSUM(32, 64) ----
            KB = KBLK * BATCH  # 32
            for q in range(NQ):
                jq = SLICES[q]
                o_ps = ps_main.tile([KB, jq, S], f32, tag="ops")
                for jj in range(jq):
                    j = SOFF[q] + jj   # core-local j
                    h, jh = j // JH, j % JH
                    # cos-term matmuls first: they only need cs, which lands
                    # one DMA earlier than ss
                    for c in range(NCHUNK):
                        nc.tensor.matmul(o_ps[:, jj, :],
                                         wc_t[c][h][:, :, :, jh],
                                         cs_sb[q][:, c, jj, :],
                                         start=(c == 0), stop=False)
                    for c in range(NCHUNK):
                        nc.tensor.matmul(o_ps[:, jj, :],
                                         ws_t[c][h][:, :, :, jh],
                                         ss_sb[q][:, c, jj, :],
                                         start=False, stop=(c == NCHUNK - 1))
                o_sb = outp.tile([KB, jq, S], f32, tag=f"osb{q % 2}")
                if q % 2 == 0:
                    nc.scalar.copy(o_sb[:], o_ps[:])
                else:
                    nc.vector.tensor_copy(o_sb[:], o_ps[:])
                nc.sync.dma_start(out_d[q], o_sb[:])

    nc.compile()
    return nc


def _input_maps(x):
    aT, bT, cs, ss, ck, sk = _cache["consts"]
    xr = np.ascontiguousarray(x.transpose(1, 0, 2))  # (n, b, m)
    in_maps = []
    for core in range(NCORES):
        jsl = slice(core * JPER, (core + 1) * JPER)
        m = {
            "x": xr.astype(np.float16),
            "aT": aT.astype(np.float16),
            "bT": np.ascontiguousarray(bT[:, jsl]).astype(np.float16),
            "ck": np.ascontiguousarray(ck[:, :, :, jsl]),
            "sk": np.ascontiguousarray(sk[:, :, :, jsl]),
        }
        for q in range(NQ):
            jq = slice(core * JPER + SOFF[q], core * JPER + SOFF[q] + SLICES[q])
            m[f"cs{q}"] = np.ascontiguousarray(cs[:, :, jq, :])
            m[f"ss{q}"] = np.ascontiguousarray(ss[:, :, jq, :])
        in_maps.append(m)
    return in_maps


def kernel(x, chi, tau):
    from concourse.bass_utils import run_bass_kernel_spmd

    x = np.asarray(x, dtype=np.float32).reshape(BATCH, NZ, NX)
    chi = float(np.asarray(chi))
    assert abs(chi - CHI) < 1e-6, "kernel compiled for chi=0.03"

    if "prog" not in _cache:
        _cache["consts"] = _host_constants()
        _cache["prog"] = _build_program()
    nc = _cache["prog"]

    res = run_bass_kernel_spmd(nc, _input_maps(x), core_ids=list(range(NCORES)))
    _cache["last_exec_ns"] = res.exec_time_ns

    out = np.empty((BATCH, 1, T, LX), dtype=np.float32)
    for core in range(NCORES):
        for q in range(NQ):
            jsl = slice(core * JPER + SOFF[q],
                        core * JPER + SOFF[q] + SLICES[q])
            r = res.results[core][f"out{q}"].reshape(KBLK, BATCH, SLICES[q], S)
            # r[k, b, jj, s] -> out[b, 64k+s, j]
            out[:, 0, :, jsl] = (r.transpose(1, 0, 3, 2)
                                 .reshape(BATCH, T, SLICES[q]))
    return out


def last_exec_time_ns():
    return _cache.get("last_exec_ns")


def sim_time_ns():
    """TimelineSim estimate for one core (used when HW profiling is absent)."""
    if "sim_ns" not in _cache:
        if "prog" not in _cache:
            _cache["consts"] = _host_constants()
            _cache["prog"] = _build_program()
        from concourse.timeline_sim import TimelineSim
        _cache["sim_ns"] = int(TimelineSim(_cache["prog"], trace=False).simulate())
    return _cache["sim_ns"]


# revision 9
# speedup vs baseline: 1.0023x; 1.0023x over previous
"""Trainium2 Bass kernel for nn_PhysicsForwardModel — table-factorized version.

Math (reference):
  W[b]       = A2 @ x_b @ Bm^T         A2 = diag(v) @ Dy[:, :NZ] / 3
  att[t,i,j] = exp(-chi*F[i,j]*t) * cos(F[i,j]*t)
  out[b,t,j] = sum_i att[t,i,j] * W[b,i,j]

Split t = S*k + s (S=64, k<8).  Angle addition gives
  att[t] = Ck * cs_s - Sk * ss_s
with x-independent tables (precomputed on host, fp16, DMA'd in):
  cs[i,j,s] = e^{-chi F s} cos(F s)     ss[i,j,s] = e^{-chi F s} sin(F s)
  Ck[i,j]   = e^{-chi F S k} cos(F S k) Sk[i,j]  = -e^{-chi F S k} sin(F S k)
Folding the k-level into the matmul stationary:
  out[b, S*k+s, j] = sum_i (W*Ck)[b,i,j] cs[i,j,s] + sum_i (W*Sk)[b,i,j] ss[i,j,s]
so the device does: W build (PE), WC/WS = W*coeff (DVE, fp16), then per
(c-chunk, j) two accumulating matmuls with stationary (128, 8k*4b) and moving
table slice (128, 64) — all transcendentals are gone from the device.

Sharding: j (=LX) split across 8 cores, 64 columns each.  Tables are DMA'd in
j-slices so the matmul loop pipelines under the table transfer (the kernel is
HBM-bandwidth-bound on the ~9.5MB/core of tables); the final slices are
smaller to shorten the end-of-stream tail.
Output per core: per-slice (32=(k,b), jq, 64 s) f32; host unscrambles.
"""

import numpy as np

BATCH, NZ, NX = 4, 128, 128
LY = LX = 512
T = LY
NCORES = 8
JPER = LX // NCORES   # 64
NCHUNK = LY // 128    # 4 i-chunks
S = 64                # timesteps per block
KBLK = T // S         # 8 blocks
# j-slice sizes for DMA/compute pipelining; final slices smaller to cut the
# end-of-stream latency tail.  One PSUM group per slice.
SLICES = [8, 8, 8, 8, 8, 8, 8, 4, 4]
SOFF = [sum(SLICES[:i]) for i in range(len(SLICES))]
NQ = len(SLICES)
CHI = float(np.float32(0.03))

_cache = {}


def _dct_mat(N):
    n = np.arange(N, dtype=np.float64)
    D = np.cos(np.pi * (2.0 * n[None, :] + 1.0) * n[:, None] / (2.0 * N))
    s = np.where(np.arange(N) == 0, np.sqrt(1.0 / N), np.sqrt(2.0 / N))
    return s[:, None] * D


def _host_constants():
    Dy = _dct_mat(LY)
    Dx = _dct_mat(LX)
    A = Dy[:, :NZ]                      # (LY, NZ)
    v = Dy @ Dy[:, 0]                   # (LY,)
    A2 = (v[:, None] * A) / 3.0         # (LY, NZ)
    Bm = Dx[:, NX:2 * NX]               # (LX, NX)
    kx = np.arange(LX, dtype=np.float64) / LX * np.pi
    ky = np.arange(LY, dtype=np.float64) / LY * np.pi
    F = np.sqrt(kx[None, :] ** 2 + ky[:, None] ** 2)  # (LY i, LX j)

    aT = A2.T.astype(np.float32)        # (128 n, 512 i)
    bT = Bm.T.astype(np.float32)        # (128 m, 512 j)

    # per-(i,j) tables; i -> (c, p) with i = 128c + p
    s_ = np.arange(S, dtype=np.float64)
    tk = np.arange(KBLK, dtype=np.float64) * S
    Fc = F.reshape(NCHUNK, 128, LX).transpose(1, 0, 2)     # (128 p, 4 c, 512 j)
    cs = np.exp(-CHI * Fc[..., None] * s_) * np.cos(Fc[..., None] * s_)
    ss = np.exp(-CHI * Fc[..., None] * s_) * np.sin(Fc[..., None] * s_)
    ck = np.exp(-CHI * Fc[..., None] * tk) * np.cos(Fc[..., None] * tk)
    sk = -np.exp(-CHI * Fc[..., None] * tk) * np.sin(Fc[..., None] * tk)
    # cs/ss: (128, 4, 512 j, 64 s); ck/sk -> (128, 4, 8 k, 512 j)
    ck = ck.transpose(0, 1, 3, 2)
    sk = sk.transpose(0, 1, 3, 2)
    return (aT, bT,
            cs.astype(np.float16), ss.astype(np.float16),
            ck.astype(np.float16), sk.astype(np.float16))


def _build_program():
    import concourse.tile as tile
    from concourse import bacc, mybir

    f32 = mybir.dt.float32
    f16 = mybir.dt.float16

    nc = bacc.Bacc("TRN2", target_bir_lowering=False, debug=False)

    x_d = nc.dram_tensor("x", (NZ, BATCH, NX), f16, kind="ExternalInput").ap()
    aT_d = nc.dram_tensor("aT", (NZ, LY), f16, kind="ExternalInput").ap()
    bT_d = nc.dram_tensor("bT", (NX, JPER), f16, kind="ExternalInput").ap()
    ck_d = nc.dram_tensor("ck", (128, NCHUNK, KBLK, JPER), f16,
                          kind="ExternalInput").ap()
    sk_d = nc.dram_tensor("sk", (128, NCHUNK, KBLK, JPER), f16,
                          kind="ExternalInput").ap()
    cs_d = [nc.dram_tensor(f"cs{q}", (128, NCHUNK, SLICES[q], S), f16,
                           kind="ExternalInput").ap() for q in range(NQ)]
    ss_d = [nc.dram_tensor(f"ss{q}", (128, NCHUNK, SLICES[q], S), f16,
                           kind="ExternalInput").ap() for q in range(NQ)]
    # out{q}[p=(4k+b), j_in_slice, s] -> host unscrambles
    out_d = [nc.dram_tensor(f"out{q}", (KBLK * BATCH, SLICES[q], S), f32,
                            kind="ExternalOutput").ap() for q in range(NQ)]

    with tile.TileContext(nc) as tc:
        with tc.tile_pool(name="singles", bufs=1) as singles, \
             tc.tile_pool(name="wbuild", bufs=2) as wbuild, \
             tc.tile_pool(name="outp", bufs=8) as outp, \
             tc.tile_pool(name="ps_w", bufs=1, space="PSUM") as ps_w, \
             tc.tile_pool(name="ps_main", bufs=6, space="PSUM") as ps_main:

            # ---- input DMAs: big table stream first, smalls interleaved ----
            x_sb = singles.tile([NZ, BATCH, NX], f16)
            aT_sb = singles.tile([NZ, LY], f16)
            bT_sb = singles.tile([NX, JPER], f16)
            ck_sb = singles.tile([128, NCHUNK, KBLK, JPER], f16)
            sk_sb = singles.tile([128, NCHUNK, KBLK, JPER], f16)
            cs_sb = [singles.tile([128, NCHUNK, SLICES[q], S], f16,
                                  name=f"cs{q}") for q in range(NQ)]
            ss_sb = [singles.tile([128, NCHUNK, SLICES[q], S], f16,
                                  name=f"ss{q}") for q in range(NQ)]
            nc.sync.dma_start(x_sb[:], x_d)
            nc.sync.dma_start(aT_sb[:], aT_d)
            nc.sync.dma_start(bT_sb[:], bT_d)
            nc.sync.dma_start(cs_sb[0][:], cs_d[0])
            nc.sync.dma_start(ss_sb[0][:], ss_d[0])
            nc.sync.dma_start(ck_sb[:], ck_d)
            nc.sync.dma_start(sk_sb[:], sk_d)
            for q in range(1, NQ):
                nc.sync.dma_start(cs_sb[q][:], cs_d[q])
                nc.sync.dma_start(ss_sb[q][:], ss_d[q])

            # ---- W build: w_c[c][p_i, b, j] = W[b, 128c+p_i, j] (fp16) ----
            w_c = [singles.tile([128, BATCH, JPER], f16, name=f"w{c}")
                   for c in range(NCHUNK)]
            for b in range(BATCH):
                mm1_ps = ps_w.tile([128, LY], f32, tag="mm1")
                # mm1_ps[m, i] = sum_n x[b,n,m] * aT[n,i]  ( = (A2 @ x_b)^T )
                nc.tensor.matmul(mm1_ps[:], x_sb[:, b, :], aT_sb[:],
                                 start=True, stop=True)
                mm1_sb = wbuild.tile([128, LY], f16)
                nc.scalar.copy(mm1_sb[:], mm1_ps[:])
                for c in range(NCHUNK):
                    w_ps = ps_w.tile([128, JPER], f32, tag="wps")
                    # w_ps[i, j] = sum_m mm1_sb[m, 128c+i] * bT[m, j]
                    nc.tensor.matmul(w_ps[:], mm1_sb[:, c * 128:(c + 1) * 128],
                                     bT_sb[:], start=True, stop=True)
                    nc.vector.tensor_copy(w_c[c][:, b, :], w_ps[:])

            # ---- stationaries, per (c, j-quarter) so PE can start early ----
            # wc[c][h][p, k, b, jj] = w * ck ;  ws = w * sk
            JH = JPER // 4  # 16
            wc_t = [[singles.tile([128, KBLK, BATCH, JH], f16, name=f"wc{c}_{h}")
                     for h in range(4)] for c in range(NCHUNK)]
            ws_t = [[singles.tile([128, KBLK, BATCH, JH], f16, name=f"ws{c}_{h}")
                     for h in range(4)] for c in range(NCHUNK)]
            for h in range(4):
                jsl = slice(h * JH, (h + 1) * JH)
                for c in range(NCHUNK):
                    nc.vector.tensor_tensor(
                        wc_t[c][h][:],
                        w_c[c][:, None, :, jsl].to_broadcast(
                            (128, KBLK, BATCH, JH)),
                        ck_sb[:, c, :, None, jsl].to_broadcast(
                            (128, KBLK, BATCH, JH)),
                        mybir.AluOpType.mult)
                    nc.vector.tensor_tensor(
                        ws_t[c][h][:],
                        w_c[c][:, None, :, jsl].to_broadcast(
                            (128, KBLK, BATCH, JH)),
                        sk_sb[:, c, :, None, jsl].to_broadcast(
                            (128, KBLK, BATCH, JH)),
                        mybir.AluOpType.mult)

            # ---- main loop: per j, 8 accumulating matmuls into PSUM(32, 64) ----
            KB = KBLK * BATCH  # 32
            for q in range(NQ):
                jq = SLICES[q]
                o_ps = ps_main.tile([KB, jq, S], f32, tag="ops")
                for jj in range(jq):
                    j = SOFF[q] + jj   # core-local j
                    h, jh = j // JH, j % JH
                    # cos-term matmuls first: they only need cs, which lands
                    # one DMA earlier than ss
                    for c in range(NCHUNK):
                        nc.tensor.matmul(o_ps[:, jj, :],
                                         wc_t[c][h][:, :, :, jh],
                                         cs_sb[q][:, c, jj, :],
                                         start=(c == 0), stop=False)
                    for c in range(NCHUNK):
                        nc.tensor.matmul(o_ps[:, jj, :],
                                         ws_t[c][h][:, :, :, jh],
                                         ss_sb[q][:, c, jj, :],
                                         start=False, stop=(c == NCHUNK - 1))
                o_sb = outp.tile([KB, jq, S], f32, tag=f"osb{q % 2}")
                if q % 2 == 0:
                    nc.scalar.copy(o_sb[:], o_ps[:])
                else:
                    nc.vector.tensor_copy(o_sb[:], o_ps[:])
                nc.sync.dma_start(out_d[q], o_sb[:])

    nc.compile()
    return nc


def _static_maps():
    """x-independent per-core inputs (tables); built once and cached."""
    aT, bT, cs, ss, ck, sk = _cache["consts"]
    maps = []
    for core in range(NCORES):
        jsl = slice(core * JPER, (core + 1) * JPER)
        m = {
            "aT": aT.astype(np.float16),
            "bT": np.ascontiguousarray(bT[:, jsl]).astype(np.float16),
            "ck": np.ascontiguousarray(ck[:, :, :, jsl]),
            "sk": np.ascontiguousarray(sk[:, :, :, jsl]),
        }
        for q in range(NQ):
            jq = slice(core * JPER + SOFF[q], core * JPER + SOFF[q] + SLICES[q])
            m[f"cs{q}"] = np.ascontiguousarray(cs[:, :, jq, :])
            m[f"ss{q}"] = np.ascontiguousarray(ss[:, :, jq, :])
        maps.append(m)
    return maps


def _input_maps(x):
    if "static_maps" not in _cache:
        _cache["static_maps"] = _static_maps()
    xr = np.ascontiguousarray(x.transpose(1, 0, 2)).astype(np.float16)  # (n,b,m)
    return [{**m, "x": xr} for m in _cache["static_maps"]]


def kernel(x, chi, tau):
    from concourse.bass_utils import run_bass_kernel_spmd

    x = np.asarray(x, dtype=np.float32).reshape(BATCH, NZ, NX)
    chi = float(np.asarray(chi))
    assert abs(chi - CHI) < 1e-6, "kernel compiled for chi=0.03"

    if "prog" not in _cache:
        _cache["consts"] = _host_constants()
        _cache["prog"] = _build_program()
    nc = _cache["prog"]

    res = run_bass_kernel_spmd(nc, _input_maps(x), core_ids=list(range(NCORES)))
    _cache["last_exec_ns"] = res.exec_time_ns

    out = np.empty((BATCH, 1, T, LX), dtype=np.float32)
    for core in range(NCORES):
        for q in range(NQ):
            jsl = slice(core * JPER + SOFF[q],
                        core * JPER + SOFF[q] + SLICES[q])
            r = res.results[core][f"out{q}"].reshape(KBLK, BATCH, SLICES[q], S)
            # r[k, b, jj, s] -> out[b, 64k+s, j]
            out[:, 0, :, jsl] = (r.transpose(1, 0, 3, 2)
                                 .reshape(BATCH, T, SLICES[q]))
    return out


def last_exec_time_ns():
    return _cache.get("last_exec_ns")


def sim_time_ns():
    """TimelineSim estimate for one core (used when HW profiling is absent)."""
    if "sim_ns" not in _cache:
        if "prog" not in _cache:
            _cache["consts"] = _host_constants()
            _cache["prog"] = _build_program()
        from concourse.timeline_sim import TimelineSim
        _cache["sim_ns"] = int(TimelineSim(_cache["prog"], trace=False).simulate())
    return _cache["sim_ns"]


# revision 10
# speedup vs baseline: 1.2357x; 1.2329x over previous
"""Trainium2 Bass kernel for nn_PhysicsForwardModel — table-factorized version.

Math (reference):
  W[b]       = A2 @ x_b @ Bm^T         A2 = diag(v) @ Dy[:, :NZ] / 3
  att[t,i,j] = exp(-chi*F[i,j]*t) * cos(F[i,j]*t)
  out[b,t,j] = sum_i att[t,i,j] * W[b,i,j]

Split t = S*k + s (S=44, k<12; last block ragged).  Angle addition gives
  att[t] = Ck * cs_s - Sk * ss_s
with x-independent tables (precomputed on host, fp16, DMA'd in):
  cs[i,j,s] = e^{-chi F s} cos(F s)     ss[i,j,s] = e^{-chi F s} sin(F s)
  Ck[i,j]   = e^{-chi F S k} cos(F S k) Sk[i,j]  = -e^{-chi F S k} sin(F S k)
Folding the k-level into the matmul stationary:
  out[b, S*k+s, j] = sum_i (W*Ck)[b,i,j] cs[i,j,s] + sum_i (W*Sk)[b,i,j] ss[i,j,s]
so the device does: W build (PE), WC/WS = W*coeff (DVE, fp16), then per
(c-chunk, j) accumulating matmuls with stationary (128, 12k*4b) and moving
table slice (128, 44) — all transcendentals are gone from the device.
S trades DMA bytes (tables scale as S + T/S) against the DVE stationary-prep
chain (scales as T/S); S=44 balances them.

Sharding: j (=LX) split across 8 cores, 64 columns each.  Tables are DMA'd in
j-slices so the matmul loop pipelines under the table transfer (the kernel is
HBM-bandwidth-bound on the ~7.4MB/core of tables); the final slices are
smaller to shorten the end-of-stream tail.
Output per core: per-slice (32=(k,b), jq, 64 s) f32; host unscrambles.
"""

import numpy as np

BATCH, NZ, NX = 4, 128, 128
LY = LX = 512
T = LY
NCORES = 8
JPER = LX // NCORES   # 64
NCHUNK = LY // 128    # 4 i-chunks
S = 44                # timesteps per block (last block ragged: 512 = 44*11+28)
KBLK = 12             # blocks; block k covers t in [44k, min(44k+44, 512))
SLAST = T - S * (KBLK - 1)  # 32 valid s in the last block
# j-slice sizes for DMA/compute pipelining; final slices smaller to cut the
# end-of-stream latency tail.  One PSUM group per slice.
SLICES = [11, 11, 11, 11, 10, 6, 4]
SOFF = [sum(SLICES[:i]) for i in range(len(SLICES))]
NQ = len(SLICES)
CHI = float(np.float32(0.03))

_cache = {}


def _dct_mat(N):
    n = np.arange(N, dtype=np.float64)
    D = np.cos(np.pi * (2.0 * n[None, :] + 1.0) * n[:, None] / (2.0 * N))
    s = np.where(np.arange(N) == 0, np.sqrt(1.0 / N), np.sqrt(2.0 / N))
    return s[:, None] * D


def _host_constants():
    Dy = _dct_mat(LY)
    Dx = _dct_mat(LX)
    A = Dy[:, :NZ]                      # (LY, NZ)
    v = Dy @ Dy[:, 0]                   # (LY,)
    A2 = (v[:, None] * A) / 3.0         # (LY, NZ)
    Bm = Dx[:, NX:2 * NX]               # (LX, NX)
    kx = np.arange(LX, dtype=np.float64) / LX * np.pi
    ky = np.arange(LY, dtype=np.float64) / LY * np.pi
    F = np.sqrt(kx[None, :] ** 2 + ky[:, None] ** 2)  # (LY i, LX j)

    aT = A2.T.astype(np.float32)        # (128 n, 512 i)
    bT = Bm.T.astype(np.float32)        # (128 m, 512 j)

    # per-(i,j) tables; i -> (c, p) with i = 128c + p
    s_ = np.arange(S, dtype=np.float64)
    tk = np.arange(KBLK, dtype=np.float64) * S
    Fc = F.reshape(NCHUNK, 128, LX).transpose(1, 0, 2)     # (128 p, 4 c, 512 j)
    cs = np.exp(-CHI * Fc[..., None] * s_) * np.cos(Fc[..., None] * s_)
    ss = np.exp(-CHI * Fc[..., None] * s_) * np.sin(Fc[..., None] * s_)
    ck = np.exp(-CHI * Fc[..., None] * tk) * np.cos(Fc[..., None] * tk)
    sk = -np.exp(-CHI * Fc[..., None] * tk) * np.sin(Fc[..., None] * tk)
    # cs/ss: (128, 4, 512 j, 64 s); ck/sk -> (128, 4, 8 k, 512 j)
    ck = ck.transpose(0, 1, 3, 2)
    sk = sk.transpose(0, 1, 3, 2)
    return (aT, bT,
            cs.astype(np.float16), ss.astype(np.float16),
            ck.astype(np.float16), sk.astype(np.float16))


def _build_program():
    import concourse.tile as tile
    from concourse import bacc, mybir

    f32 = mybir.dt.float32
    f16 = mybir.dt.float16

    nc = bacc.Bacc("TRN2", target_bir_lowering=False, debug=False)

    PK = BATCH * NX + LY + JPER  # 1088: packed [x | aT | bT]
    pk_d = nc.dram_tensor("pk", (NZ, PK), f16, kind="ExternalInput").ap()
    ck_d = nc.dram_tensor("ck", (128, NCHUNK, KBLK, JPER), f16,
                          kind="ExternalInput").ap()
    sk_d = nc.dram_tensor("sk", (128, NCHUNK, KBLK, JPER), f16,
                          kind="ExternalInput").ap()
    cs_d = [nc.dram_tensor(f"cs{q}", (128, NCHUNK, SLICES[q], S), f16,
                           kind="ExternalInput").ap() for q in range(NQ)]
    ss_d = [nc.dram_tensor(f"ss{q}", (128, NCHUNK, SLICES[q], S), f16,
                           kind="ExternalInput").ap() for q in range(NQ)]
    # out{q}[p=(4k+b), j_in_slice, s] -> host unscrambles; the last two
    # slices share one output tensor/DMA to cut tail trigger+descgen count
    NOUT = NQ - 1
    OSL = SLICES[:NQ - 2] + [SLICES[NQ - 2] + SLICES[NQ - 1]]
    out_d = [nc.dram_tensor(f"out{q}", (KBLK * BATCH, OSL[q], S), f16,
                            kind="ExternalOutput").ap() for q in range(NOUT)]

    with tile.TileContext(nc) as tc:
        with tc.tile_pool(name="singles", bufs=1) as singles, \
             tc.tile_pool(name="wbuild", bufs=2) as wbuild, \
             tc.tile_pool(name="outp", bufs=8) as outp, \
             tc.tile_pool(name="ps_w", bufs=2, space="PSUM") as ps_w, \
             tc.tile_pool(name="ps_w2", bufs=2, space="PSUM") as ps_w2, \
             tc.tile_pool(name="ps_main", bufs=4, space="PSUM") as ps_main:

            # ---- input DMAs: packed smalls, then the table stream ----
            pk_sb = singles.tile([NZ, PK], f16)
            ck_sb = singles.tile([128, NCHUNK, KBLK, JPER], f16)
            sk_sb = singles.tile([128, NCHUNK, KBLK, JPER], f16)
            cs_sb = [singles.tile([128, NCHUNK, SLICES[q], S], f16,
                                  name=f"cs{q}") for q in range(NQ)]
            ss_sb = [singles.tile([128, NCHUNK, SLICES[q], S], f16,
                                  name=f"ss{q}") for q in range(NQ)]
            nc.sync.dma_start(pk_sb[:], pk_d)
            nc.sync.dma_start(ck_sb[:], ck_d)
            nc.sync.dma_start(sk_sb[:], sk_d)
            nc.sync.dma_start(cs_sb[0][:], cs_d[0])
            nc.sync.dma_start(ss_sb[0][:], ss_d[0])
            for q in range(1, NQ):
                nc.sync.dma_start(cs_sb[q][:], cs_d[q])
                nc.sync.dma_start(ss_sb[q][:], ss_d[q])

            # ---- W build: w_all[p_i, c, b, j] = W[b, 128c+p_i, j] (fp16) ----
            w_all = singles.tile([128, NCHUNK, BATCH, JPER], f16)
            for b in range(BATCH):
                mm1_ps = ps_w.tile([128, LY], f32, tag="mm1")
                # mm1_ps[m, i] = sum_n x[b,n,m] * aT[n,i]  ( = (A2 @ x_b)^T )
                nc.tensor.matmul(mm1_ps[:], pk_sb[:, b * NX:(b + 1) * NX],
                                 pk_sb[:, BATCH * NX:BATCH * NX + LY],
                                 start=True, stop=True)
                mm1_sb = wbuild.tile([128, LY], f16)
                nc.scalar.copy(mm1_sb[:], mm1_ps[:])
                w2_ps = ps_w2.tile([128, NCHUNK, JPER], f32, tag="wps")
                for c in range(NCHUNK):
                    # w2_ps[i, c, j] = sum_m mm1_sb[m, 128c+i] * bT[m, j]
                    nc.tensor.matmul(w2_ps[:, c, :],
                                     mm1_sb[:, c * 128:(c + 1) * 128],
                                     pk_sb[:, BATCH * NX + LY:],
                                     start=True, stop=True)
                nc.vector.tensor_copy(w_all[:, :, b, :], w2_ps[:])

            # ---- stationaries, per (c, j-quarter) so PE can start early ----
            # wc[c][h][p, k, b, jj] = w * ck ;  ws = w * sk
            JH = JPER // 4  # 16
            wc_t = [[singles.tile([128, KBLK, BATCH, JH], f16, name=f"wc{c}_{h}")
                     for h in range(4)] for c in range(NCHUNK)]
            ws_t = [[singles.tile([128, KBLK, BATCH, JH], f16, name=f"ws{c}_{h}")
                     for h in range(4)] for c in range(NCHUNK)]
            for h in range(4):
                jsl = slice(h * JH, (h + 1) * JH)
                for c in range(NCHUNK):
                    nc.vector.tensor_tensor(
                        wc_t[c][h][:],
                        w_all[:, c, None, :, jsl].to_broadcast(
                            (128, KBLK, BATCH, JH)),
                        ck_sb[:, c, :, None, jsl].to_broadcast(
                            (128, KBLK, BATCH, JH)),
                        mybir.AluOpType.mult)
                    nc.vector.tensor_tensor(
                        ws_t[c][h][:],
                        w_all[:, c, None, :, jsl].to_broadcast(
                            (128, KBLK, BATCH, JH)),
                        sk_sb[:, c, :, None, jsl].to_broadcast(
                            (128, KBLK, BATCH, JH)),
                        mybir.AluOpType.mult)

            # ---- main loop: per j, 8 accumulating matmuls into PSUM(32, 64) ----
            KB = KBLK * BATCH  # 44
            for q in range(NQ):
                jq = SLICES[q]
                o_ps = ps_main.tile([KB, jq, S], f32, tag="ops")
                for jj in range(jq):
                    j = SOFF[q] + jj   # core-local j
                    h, jh = j // JH, j % JH
                    # cos-term matmuls first: they only need cs, which lands
                    # one DMA earlier than ss
                    for c in range(NCHUNK):
                        nc.tensor.matmul(o_ps[:, jj, :],
                                         wc_t[c][h][:, :, :, jh],
                                         cs_sb[q][:, c, jj, :],
                                         start=(c == 0), stop=False)
                    for c in range(NCHUNK):
                        nc.tensor.matmul(o_ps[:, jj, :],
                                         ws_t[c][h][:, :, :, jh],
                                         ss_sb[q][:, c, jj, :],
                                         start=False, stop=(c == NCHUNK - 1))
                if q < NQ - 2:
                    o_sb = outp.tile([KB, jq, S], f16, tag=f"osb{q % 2}")
                    nc.scalar.copy(o_sb[:], o_ps[:])
                    nc.sync.dma_start(out_d[q], o_sb[:])
                elif q == NQ - 2:
                    o_last = outp.tile([KB, OSL[-1], S], f16, tag="olast")
                    nc.vector.tensor_copy(o_last[:, :jq, :], o_ps[:])
                else:
                    # DVE is idle once its TT chain drains; final copies on it
                    nc.vector.tensor_copy(o_last[:, OSL[-1] - jq:, :], o_ps[:])
                    nc.sync.dma_start(out_d[NOUT - 1], o_last[:])

    nc.compile()
    return nc


def _static_maps():
    """x-independent per-core inputs (tables); built once and cached."""
    aT, bT, cs, ss, ck, sk = _cache["consts"]
    maps = []
    abt = []
    for core in range(NCORES):
        jsl = slice(core * JPER, (core + 1) * JPER)
        m = {
            "ck": np.ascontiguousarray(ck[:, :, :, jsl]),
            "sk": np.ascontiguousarray(sk[:, :, :, jsl]),
        }
        for q in range(NQ):
            jq = slice(core * JPER + SOFF[q], core * JPER + SOFF[q] + SLICES[q])
            m[f"cs{q}"] = np.ascontiguousarray(cs[:, :, jq, :])
            m[f"ss{q}"] = np.ascontiguousarray(ss[:, :, jq, :])
        maps.append(m)
        abt.append(np.concatenate(
            [aT, bT[:, jsl]], axis=1).astype(np.float16))  # (128, 576)
    _cache["abt"] = abt
    return maps


def _input_maps(x):
    if "static_maps" not in _cache:
        _cache["static_maps"] = _static_maps()
    xr = np.ascontiguousarray(x.transpose(1, 0, 2)).astype(np.float16)
    xf = xr.reshape(NZ, BATCH * NX)  # (n, b*m)
    return [{**m, "pk": np.concatenate([xf, _cache["abt"][core]], axis=1)}
            for core, m in enumerate(_cache["static_maps"])]


def kernel(x, chi, tau):
    from concourse.bass_utils import run_bass_kernel_spmd

    x = np.asarray(x, dtype=np.float32).reshape(BATCH, NZ, NX)
    chi = float(np.asarray(chi))
    assert abs(chi - CHI) < 1e-6, "kernel compiled for chi=0.03"

    if "prog" not in _cache:
        _cache["consts"] = _host_constants()
        _cache["prog"] = _build_program()
    nc = _cache["prog"]

    res = run_bass_kernel_spmd(nc, _input_maps(x), core_ids=list(range(NCORES)))
    _cache["last_exec_ns"] = res.exec_time_ns

    out = np.empty((BATCH, 1, T, LX), dtype=np.float32)
    for core in range(NCORES):
        OSL = SLICES[:NQ - 2] + [SLICES[NQ - 2] + SLICES[NQ - 1]]
        for q in range(NQ - 1):
            jsl = slice(core * JPER + SOFF[q],
                        core * JPER + SOFF[q] + OSL[q])
            r = res.results[core][f"out{q}"].reshape(KBLK, BATCH, OSL[q], S)
            # r[k, b, jj, s] -> out[b, S*k+s, j]; last block only SLAST valid
            rt = r.transpose(1, 0, 3, 2)  # (b, k, s, jj)
            full = rt[:, :KBLK - 1].reshape(BATCH, S * (KBLK - 1), OSL[q])
            out[:, 0, :S * (KBLK - 1), jsl] = full
            out[:, 0, S * (KBLK - 1):, jsl] = rt[:, KBLK - 1, :SLAST]
    return out


def last_exec_time_ns():
    return _cache.get("last_exec_ns")


def sim_time_ns():
    """TimelineSim estimate for one core (used when HW profiling is absent)."""
    if "sim_ns" not in _cache:
        if "prog" not in _cache:
            _cache["consts"] = _host_constants()
            _cache["prog"] = _build_program()
        from concourse.timeline_sim import TimelineSim
        _cache["sim_ns"] = int(TimelineSim(_cache["prog"], trace=False).simulate())
    return _cache["sim_ns"]


# revision 11
# speedup vs baseline: 1.2381x; 1.0020x over previous
"""Trainium2 Bass kernel for nn_PhysicsForwardModel — table-factorized version.

Math (reference):
  W[b]       = A2 @ x_b @ Bm^T         A2 = diag(v) @ Dy[:, :NZ] / 3
  att[t,i,j] = exp(-chi*F[i,j]*t) * cos(F[i,j]*t)
  out[b,t,j] = sum_i att[t,i,j] * W[b,i,j]

Split t = S*k + s (S=44, k<12; last block ragged).  Angle addition gives
  att[t] = Ck * cs_s - Sk * ss_s
with x-independent tables (precomputed on host, fp16, DMA'd in):
  cs[i,j,s] = e^{-chi F s} cos(F s)     ss[i,j,s] = e^{-chi F s} sin(F s)
  Ck[i,j]   = e^{-chi F S k} cos(F S k) Sk[i,j]  = -e^{-chi F S k} sin(F S k)
Folding the k-level into the matmul stationary:
  out[b, S*k+s, j] = sum_i (W*Ck)[b,i,j] cs[i,j,s] + sum_i (W*Sk)[b,i,j] ss[i,j,s]
so the device does: W build (PE), WC/WS = W*coeff (DVE, fp16), then per
(c-chunk, j) accumulating matmuls with stationary (128, 12k*4b) and moving
table slice (128, 44) — all transcendentals are gone from the device.
S trades DMA bytes (tables scale as S + T/S) against the DVE stationary-prep
chain (scales as T/S); S=44 balances them.

Sharding: j (=LX) split across 8 cores, 64 columns each.  Tables are DMA'd in
j-slices so the matmul loop pipelines under the table transfer (the kernel is
HBM-bandwidth-bound on the ~7.4MB/core of tables); the final slices are
smaller to shorten the end-of-stream tail.
Output per core: per-slice (32=(k,b), jq, 64 s) f32; host unscrambles.
"""

import numpy as np

BATCH, NZ, NX = 4, 128, 128
LY = LX = 512
T = LY
NCORES = 8
JPER = LX // NCORES   # 64
NCHUNK = LY // 128    # 4 i-chunks
S = 44                # timesteps per block (last block ragged: 512 = 44*11+28)
KBLK = 12             # blocks; block k covers t in [44k, min(44k+44, 512))
SLAST = T - S * (KBLK - 1)  # 32 valid s in the last block
# j-slice sizes for DMA/compute pipelining; final slices smaller to cut the
# end-of-stream latency tail.  One PSUM group per slice.
SLICES = [11, 11, 11, 11, 11, 6, 3]
SOFF = [sum(SLICES[:i]) for i in range(len(SLICES))]
NQ = len(SLICES)
CHI = float(np.float32(0.03))

_cache = {}


def _dct_mat(N):
    n = np.arange(N, dtype=np.float64)
    D = np.cos(np.pi * (2.0 * n[None, :] + 1.0) * n[:, None] / (2.0 * N))
    s = np.where(np.arange(N) == 0, np.sqrt(1.0 / N), np.sqrt(2.0 / N))
    return s[:, None] * D


def _host_constants():
    Dy = _dct_mat(LY)
    Dx = _dct_mat(LX)
    A = Dy[:, :NZ]                      # (LY, NZ)
    v = Dy @ Dy[:, 0]                   # (LY,)
    A2 = (v[:, None] * A) / 3.0         # (LY, NZ)
    Bm = Dx[:, NX:2 * NX]               # (LX, NX)
    kx = np.arange(LX, dtype=np.float64) / LX * np.pi
    ky = np.arange(LY, dtype=np.float64) / LY * np.pi
    F = np.sqrt(kx[None, :] ** 2 + ky[:, None] ** 2)  # (LY i, LX j)

    aT = A2.T.astype(np.float32)        # (128 n, 512 i)
    bT = Bm.T.astype(np.float32)        # (128 m, 512 j)

    # per-(i,j) tables; i -> (c, p) with i = 128c + p
    s_ = np.arange(S, dtype=np.float64)
    tk = np.arange(KBLK, dtype=np.float64) * S
    Fc = F.reshape(NCHUNK, 128, LX).transpose(1, 0, 2)     # (128 p, 4 c, 512 j)
    cs = np.exp(-CHI * Fc[..., None] * s_) * np.cos(Fc[..., None] * s_)
    ss = np.exp(-CHI * Fc[..., None] * s_) * np.sin(Fc[..., None] * s_)
    ck = np.exp(-CHI * Fc[..., None] * tk) * np.cos(Fc[..., None] * tk)
    sk = -np.exp(-CHI * Fc[..., None] * tk) * np.sin(Fc[..., None] * tk)
    # cs/ss: (128, 4, 512 j, 64 s); ck/sk -> (128, 4, 8 k, 512 j)
    ck = ck.transpose(0, 1, 3, 2)
    sk = sk.transpose(0, 1, 3, 2)
    return (aT, bT,
            cs.astype(np.float16), ss.astype(np.float16),
            ck.astype(np.float16), sk.astype(np.float16))


def _build_program():
    import concourse.tile as tile
    from concourse import bacc, mybir

    f32 = mybir.dt.float32
    f16 = mybir.dt.float16

    nc = bacc.Bacc("TRN2", target_bir_lowering=False, debug=False)

    PK = BATCH * NX + LY + JPER  # 1088: packed [x | aT | bT]
    pk_d = nc.dram_tensor("pk", (NZ, PK), f16, kind="ExternalInput").ap()
    ck_d = nc.dram_tensor("ck", (128, NCHUNK, KBLK, JPER), f16,
                          kind="ExternalInput").ap()
    sk_d = nc.dram_tensor("sk", (128, NCHUNK, KBLK, JPER), f16,
                          kind="ExternalInput").ap()
    cs_d = [nc.dram_tensor(f"cs{q}", (128, NCHUNK, SLICES[q], S), f16,
                           kind="ExternalInput").ap() for q in range(NQ)]
    ss_d = [nc.dram_tensor(f"ss{q}", (128, NCHUNK, SLICES[q], S), f16,
                           kind="ExternalInput").ap() for q in range(NQ)]
    # out{q}[p=(4k+b), j_in_slice, s] -> host unscrambles; the last two
    # slices share one output tensor/DMA to cut tail trigger+descgen count
    NOUT = NQ - 1
    OSL = SLICES[:NQ - 2] + [SLICES[NQ - 2] + SLICES[NQ - 1]]
    out_d = [nc.dram_tensor(f"out{q}", (KBLK * BATCH, OSL[q], S), f16,
                            kind="ExternalOutput").ap() for q in range(NOUT)]

    with tile.TileContext(nc) as tc:
        with tc.tile_pool(name="singles", bufs=1) as singles, \
             tc.tile_pool(name="wbuild", bufs=2) as wbuild, \
             tc.tile_pool(name="outp", bufs=8) as outp, \
             tc.tile_pool(name="ps_w", bufs=2, space="PSUM") as ps_w, \
             tc.tile_pool(name="ps_w2", bufs=2, space="PSUM") as ps_w2, \
             tc.tile_pool(name="ps_main", bufs=4, space="PSUM") as ps_main:

            # ---- input DMAs: packed smalls, then the table stream ----
            pk_sb = singles.tile([NZ, PK], f16)
            ck_sb = singles.tile([128, NCHUNK, KBLK, JPER], f16)
            sk_sb = singles.tile([128, NCHUNK, KBLK, JPER], f16)
            cs_sb = [singles.tile([128, NCHUNK, SLICES[q], S], f16,
                                  name=f"cs{q}") for q in range(NQ)]
            ss_sb = [singles.tile([128, NCHUNK, SLICES[q], S], f16,
                                  name=f"ss{q}") for q in range(NQ)]
            nc.sync.dma_start(pk_sb[:], pk_d)
            nc.sync.dma_start(ck_sb[:], ck_d)
            nc.sync.dma_start(sk_sb[:], sk_d)
            nc.sync.dma_start(cs_sb[0][:], cs_d[0])
            nc.sync.dma_start(ss_sb[0][:], ss_d[0])
            for q in range(1, NQ):
                nc.sync.dma_start(cs_sb[q][:], cs_d[q])
                nc.sync.dma_start(ss_sb[q][:], ss_d[q])

            # ---- W build: w_all[p_i, c, b, j] = W[b, 128c+p_i, j] (fp16) ----
            w_all = singles.tile([128, NCHUNK, BATCH, JPER], f16)
            for b in range(BATCH):
                mm1_ps = ps_w.tile([128, LY], f32, tag="mm1")
                # mm1_ps[m, i] = sum_n x[b,n,m] * aT[n,i]  ( = (A2 @ x_b)^T )
                nc.tensor.matmul(mm1_ps[:], pk_sb[:, b * NX:(b + 1) * NX],
                                 pk_sb[:, BATCH * NX:BATCH * NX + LY],
                                 start=True, stop=True)
                mm1_sb = wbuild.tile([128, LY], f16)
                nc.scalar.copy(mm1_sb[:], mm1_ps[:])
                w2_ps = ps_w2.tile([128, NCHUNK, JPER], f32, tag="wps")
                for c in range(NCHUNK):
                    # w2_ps[i, c, j] = sum_m mm1_sb[m, 128c+i] * bT[m, j]
                    nc.tensor.matmul(w2_ps[:, c, :],
                                     mm1_sb[:, c * 128:(c + 1) * 128],
                                     pk_sb[:, BATCH * NX + LY:],
                                     start=True, stop=True)
                nc.vector.tensor_copy(w_all[:, :, b, :], w2_ps[:])

            # ---- stationaries, per (c, j-quarter) so PE can start early ----
            # wc[c][h][p, k, b, jj] = w * ck ;  ws = w * sk
            JH = JPER // 4  # 16
            wc_t = [[singles.tile([128, KBLK, BATCH, JH], f16, name=f"wc{c}_{h}")
                     for h in range(4)] for c in range(NCHUNK)]
            ws_t = [[singles.tile([128, KBLK, BATCH, JH], f16, name=f"ws{c}_{h}")
                     for h in range(4)] for c in range(NCHUNK)]
            for h in range(4):
                jsl = slice(h * JH, (h + 1) * JH)
                for c in range(NCHUNK):
                    nc.vector.tensor_tensor(
                        wc_t[c][h][:],
                        w_all[:, c, None, :, jsl].to_broadcast(
                            (128, KBLK, BATCH, JH)),
                        ck_sb[:, c, :, None, jsl].to_broadcast(
                            (128, KBLK, BATCH, JH)),
                        mybir.AluOpType.mult)
                    nc.vector.tensor_tensor(
                        ws_t[c][h][:],
                        w_all[:, c, None, :, jsl].to_broadcast(
                            (128, KBLK, BATCH, JH)),
                        sk_sb[:, c, :, None, jsl].to_broadcast(
                            (128, KBLK, BATCH, JH)),
                        mybir.AluOpType.mult)

            # ---- main loop: per j, 8 accumulating matmuls into PSUM(32, 64) ----
            KB = KBLK * BATCH  # 44
            for q in range(NQ):
                jq = SLICES[q]
                o_ps = ps_main.tile([KB, jq, S], f32, tag="ops")
                for jj in range(jq):
                    j = SOFF[q] + jj   # core-local j
                    h, jh = j // JH, j % JH
                    # cos-term matmuls first: they only need cs, which lands
                    # one DMA earlier than ss
                    for c in range(NCHUNK):
                        nc.tensor.matmul(o_ps[:, jj, :],
                                         wc_t[c][h][:, :, :, jh],
                                         cs_sb[q][:, c, jj, :],
                                         start=(c == 0), stop=False)
                    for c in range(NCHUNK):
                        nc.tensor.matmul(o_ps[:, jj, :],
                                         ws_t[c][h][:, :, :, jh],
                                         ss_sb[q][:, c, jj, :],
                                         start=False, stop=(c == NCHUNK - 1))
                if q < NQ - 2:
                    o_sb = outp.tile([KB, jq, S], f16, tag=f"osb{q % 2}")
                    nc.scalar.copy(o_sb[:], o_ps[:])
                    nc.sync.dma_start(out_d[q], o_sb[:])
                elif q == NQ - 2:
                    o_last = outp.tile([KB, OSL[-1], S], f16, tag="olast")
                    nc.vector.tensor_copy(o_last[:, :jq, :], o_ps[:])
                else:
                    # DVE is idle once its TT chain drains; final copies on it
                    nc.vector.tensor_copy(o_last[:, OSL[-1] - jq:, :], o_ps[:])
                    nc.sync.dma_start(out_d[NOUT - 1], o_last[:])

    nc.compile()
    return nc


def _static_maps():
    """x-independent per-core inputs (tables); built once and cached."""
    aT, bT, cs, ss, ck, sk = _cache["consts"]
    maps = []
    abt = []
    for core in range(NCORES):
        jsl = slice(core * JPER, (core + 1) * JPER)
        m = {
            "ck": np.ascontiguousarray(ck[:, :, :, jsl]),
            "sk": np.ascontiguousarray(sk[:, :, :, jsl]),
        }
        for q in range(NQ):
            jq = slice(core * JPER + SOFF[q], core * JPER + SOFF[q] + SLICES[q])
            m[f"cs{q}"] = np.ascontiguousarray(cs[:, :, jq, :])
            m[f"ss{q}"] = np.ascontiguousarray(ss[:, :, jq, :])
        maps.append(m)
        abt.append(np.concatenate(
            [aT, bT[:, jsl]], axis=1).astype(np.float16))  # (128, 576)
    _cache["abt"] = abt
    return maps


def _input_maps(x):
    if "static_maps" not in _cache:
        _cache["static_maps"] = _static_maps()
    xr = np.ascontiguousarray(x.transpose(1, 0, 2)).astype(np.float16)
    xf = xr.reshape(NZ, BATCH * NX)  # (n, b*m)
    return [{**m, "pk": np.concatenate([xf, _cache["abt"][core]], axis=1)}
            for core, m in enumerate(_cache["static_maps"])]


def kernel(x, chi, tau):
    from concourse.bass_utils import run_bass_kernel_spmd

    x = np.asarray(x, dtype=np.float32).reshape(BATCH, NZ, NX)
    chi = float(np.asarray(chi))
    assert abs(chi - CHI) < 1e-6, "kernel compiled for chi=0.03"

    if "prog" not in _cache:
        _cache["consts"] = _host_constants()
        _cache["prog"] = _build_program()
    nc = _cache["prog"]

    res = run_bass_kernel_spmd(nc, _input_maps(x), core_ids=list(range(NCORES)))
    _cache["last_exec_ns"] = res.exec_time_ns

    out = np.empty((BATCH, 1, T, LX), dtype=np.float32)
    for core in range(NCORES):
        OSL = SLICES[:NQ - 2] + [SLICES[NQ - 2] + SLICES[NQ - 1]]
        for q in range(NQ - 1):
            jsl = slice(core * JPER + SOFF[q],
                        core * JPER + SOFF[q] + OSL[q])
            r = res.results[core][f"out{q}"].reshape(KBLK, BATCH, OSL[q], S)
            # r[k, b, jj, s] -> out[b, S*k+s, j]; last block only SLAST valid
            rt = r.transpose(1, 0, 3, 2)  # (b, k, s, jj)
            full = rt[:, :KBLK - 1].reshape(BATCH, S * (KBLK - 1), OSL[q])
            out[:, 0, :S * (KBLK - 1), jsl] = full
            out[:, 0, S * (KBLK - 1):, jsl] = rt[:, KBLK - 1, :SLAST]
    return out


def last_exec_time_ns():
    return _cache.get("last_exec_ns")


def sim_time_ns():
    """TimelineSim estimate for one core (used when HW profiling is absent)."""
    if "sim_ns" not in _cache:
        if "prog" not in _cache:
            _cache["consts"] = _host_constants()
            _cache["prog"] = _build_program()
        from concourse.timeline_sim import TimelineSim
        _cache["sim_ns"] = int(TimelineSim(_cache["prog"], trace=False).simulate())
    return _cache["sim_ns"]
